# revision 5
# baseline (speedup 1.0000x reference)
"""Trainium2 Bass kernel for nn_MultiHeadClassifier.

  logits[b, c] = sum_{(g,l): label_ids[g,l]==c} group_probs[b,g] *
                 (features[b] @ W[g,l] + b[g,l])

Data-parallel over batch (8 cores, 4096 rows each). Per core:
  * Host prep: sort the G*L=1024 head outputs by target class and
    first-fit-pack them into exactly 8 chunks of 128 rows (no padding);
    chunk class bands are near-disjoint — the few overlapped columns are
    handled with accumulate (start=False) scatter matmuls. Host also
    pre-expands group_probs to the packed [1024, BC] layout (pure
    replication) and pre-transposes/casts inputs to fp16.
  * GEMM1 (PE, fp16): glT[gl, b] = W^T.T @ X^T per (chunk, b-tile),
    4 accumulating K=128 matmuls.
  * Fused (DVE): wtj = (pg + bias_j) * ptx_j via scalar_tensor_tensor,
    PSUM in, fp16 SBUF out.
  * Scatter (PE, fp16): logits[b, lo:hi] = wtj^T @ S_j per 128-row
    b-slice; bands cover [0,C) contiguously, overlaps accumulate.
  * Drain (ACT): PSUM -> fp16 SBUF; out-DMA on the sync queue.
Output is fp16 on device, cast to fp32 on host.
"""
import os
import sys
import numpy as np

for _p in ("/opt/trn_rl_repo",):
    if _p not in sys.path:
        sys.path.append(_p)

import concourse.bass as bass  # noqa: E402
import concourse.tile as tile  # noqa: E402
from concourse import bacc, mybir, bass_utils  # noqa: E402
from contextlib import ExitStack  # noqa: E402

F32 = mybir.dt.float32
F16 = mybir.dt.float16

B, F, G, L, C = 32768, 512, 16, 64, 1000
NCORE = 8
BC = B // NCORE          # 4096 batch rows per core
NT = BC // 512           # 8 b-tiles of 512
KF = F // 128            # 4 feature chunks
NCH = 8                  # 8 chunks of 128 head-outputs (exact, no pad)

LAST_EXEC_NS = None


def _host_prep(W, b, label_ids):
    """Pack the GL=1024 (group,label) rows into 8 chunks of exactly 128,
    classes kept whole per chunk when possible (first-fit in sorted class
    order; splits a class only if nothing fits). Returns packed W/bias/S
    plus the scatter segment list."""
    lab = np.asarray(label_ids).reshape(-1).astype(np.int64)
    GL = lab.shape[0]
    Wflat = np.asarray(W, dtype=np.float32).reshape(GL, F)
    bflat = np.asarray(b, dtype=np.float32).reshape(GL)

    order = np.argsort(lab, kind="stable")
    classes, starts = np.unique(lab[order], return_index=True)
    starts = list(starts) + [GL]
    # queue of [class, list of gl rows]
    queue = [[int(classes[i]), list(order[starts[i]:starts[i + 1]])]
             for i in range(len(classes))]

    chunks = []          # list of list[(gl, class)]
    cur, cap = [], 128
    while queue:
        placed = False
        for qi in range(len(queue)):
            c, rows = queue[qi]
            if len(rows) <= cap:
                cur += [(gl, c) for gl in rows]
                cap -= len(rows)
                queue.pop(qi)
                placed = True
                break
        if not placed:
            c, rows = queue[0]           # split the front class
            cur += [(gl, c) for gl in rows[:cap]]
            queue[0][1] = rows[cap:]
            cap = 0
        if cap == 0:
            chunks.append(sorted(cur, key=lambda x: x[1]))
            cur, cap = [], 128
    assert not cur and len(chunks) == NCH

    chunks.sort(key=lambda ch: ch[0][1])
    # class bands and contiguous cover blocks
    blocks = []          # (blk_lo, blk_hi) per chunk, S columns span this
    segments = []        # (j, c0, c1, s_ofs, accum)
    cov = 0
    s_off = []
    off = 0
    for j, ch in enumerate(chunks):
        lo = ch[0][1]
        hi = ch[-1][1] + 1
        blk_lo = min(lo, cov)
        blk_hi = max(hi, cov)
        if j == NCH - 1:
            blk_hi = max(blk_hi, C)
        if j == 0:
            blk_lo = 0
        blocks.append((blk_lo, blk_hi))
        s_off.append(off)
        # accumulate part: columns already covered
        if blk_lo < cov:
            segments.append((j, blk_lo, min(cov, blk_hi), off, True))
        # fresh part
        if blk_hi > cov:
            segments.append((j, max(blk_lo, cov), blk_hi,
                             off + max(blk_lo, cov) - blk_lo, False))
        cov = max(cov, blk_hi)
        off += blk_hi - blk_lo
    assert cov == C, f"cover ended at {cov}"
    SSW = off

    # split segments at 512-column PSUM bank boundaries
    segs = []
    for (j, c0, c1, s0, acc) in segments:
        while c0 < c1:
            nxt = min(c1, (c0 // 512 + 1) * 512)
            segs.append((j, c0, nxt, s0, acc))
            s0 += nxt - c0
            c0 = nxt

    WT = np.zeros((F, NCH * 128), dtype=np.float16)
    biasT = np.zeros((128, NCH), dtype=np.float32)
    SS = np.zeros((128, SSW), dtype=np.float16)
    gmap = np.zeros((NCH, 128), dtype=np.int64)
    for j, ch in enumerate(chunks):
        blk_lo, _ = blocks[j]
        for r, (gl, c) in enumerate(ch):
            WT[:, j * 128 + r] = Wflat[gl]
            biasT[r, j] = bflat[gl]
            SS[r, s_off[j] + c - blk_lo] = 1.0
            gmap[j, r] = gl // L
    return dict(WT=WT, biasT=biasT, SS=SS, SSW=SSW, gmap=gmap, segs=segs)


def _build_program(SSW, segs):
    nc = bacc.Bacc("TRN2", target_bir_lowering=False, debug=False,
                   num_devices=NCORE)
    xt_d = nc.dram_tensor("xt", [F, BC], F16, kind="ExternalInput").ap()
    ptx_d = nc.dram_tensor("ptx", [NT * 128, NCH * 512], F16,
                           kind="ExternalInput").ap()
    wt_d = nc.dram_tensor("wt", [F, NCH * 128], F16, kind="ExternalInput").ap()
    bt_d = nc.dram_tensor("bt", [128, NCH], F32, kind="ExternalInput").ap()
    s_d = nc.dram_tensor("s", [128, SSW], F16, kind="ExternalInput").ap()
    out_d = nc.dram_tensor("logits", [BC, C], F16, kind="ExternalOutput").ap()

    with tile.TileContext(nc) as tc, ExitStack() as ctx:
        const = ctx.enter_context(tc.tile_pool(name="const", bufs=1))
        psG = ctx.enter_context(tc.tile_pool(name="psG", bufs=4, space="PSUM"))
        psL = ctx.enter_context(tc.tile_pool(name="psL", bufs=2, space="PSUM"))
        sbW = ctx.enter_context(tc.tile_pool(name="sbW", bufs=18))
        sbO = ctx.enter_context(tc.tile_pool(name="sbO", bufs=4))

        # W in j-halves (chunk 0-3 / 4-7 per k) so the first GEMM only
        # waits for 4 small tiles; scalar HWDGE queue.
        wts = [[None, None] for _ in range(KF)]
        for h in range(2):
            for k in range(KF):
                t_ = const.tile([128, 512], F16, name=f"w{k}_{h}",
                                tag=f"w{k}_{h}")
                nc.scalar.dma_start(t_[:],
                                    wt_d[k * 128:(k + 1) * 128,
                                         h * 512:(h + 1) * 512])
                wts[k][h] = t_
            if h == 0:
                # first X pair right behind the first W half
                pass
        # X^T in t-pair tiles per k on the sync queue
        xts = [[None] * 4 for _ in range(KF)]
        for tp in range(4):
            for k in range(KF):
                t_ = const.tile([128, 1024], F16, name=f"x{k}_{tp}",
                                tag=f"x{k}_{tp}")
                nc.sync.dma_start(t_[:],
                                  xt_d[k * 128:(k + 1) * 128,
                                       tp * 1024:(tp + 1) * 1024])
                xts[k][tp] = t_
        bts = const.tile([128, NCH], F32, name="bts", tag="bts")
        nc.scalar.dma_start(bts[:], bt_d[:])
        ss = const.tile([128, SSW], F16, name="ss", tag="ss")
        nc.scalar.dma_start(ss[:], s_d[:])
        # expanded group probs, one tile per b-tile
        ptxs = []
        for t in range(NT):
            t_ = const.tile([128, NCH * 512], F16, name=f"ptx{t}",
                            tag=f"ptx{t}")
            eng = nc.scalar if t < 4 else nc.sync
            eng.dma_start(t_[:], ptx_d[t * 128:(t + 1) * 128, :])
            ptxs.append(t_)

        for t in range(NT):
            tp, to = divmod(t, 2)
            bsl = bass.ts(to, 512)
            wtjs = []
            for j in range(NCH):
                jh, jo = divmod(j, 4)
                jsl = bass.ts(jo, 128)
                pg = psG.tile([128, 512], F32, name="pg", tag="pg")
                for k in range(KF):
                    nc.tensor.matmul(pg[:], wts[k][jh][:, jsl],
                                     xts[k][tp][:, bsl],
                                     start=(k == 0), stop=(k == KF - 1))
                wtj = sbW.tile([128, 512], F16, name="wtj", tag="wtj")
                nc.vector.scalar_tensor_tensor(
                    wtj[:], pg[:], bts[:, j:j + 1], ptxs[t][:, bass.ts(j, 512)],
                    op0=mybir.AluOpType.add, op1=mybir.AluOpType.mult)
                wtjs.append(wtj)
            for bs_i in range(4):
                pl = psL.tile([128, 1024], F32, name="pl", tag="pl")
                for (j, c0, c1, s0, acc) in segs:
                    nc.tensor.matmul(pl[:, c0:c1],
                                     wtjs[j][:, bass.ts(bs_i, 128)],
                                     ss[:, s0:s0 + (c1 - c0)],
                                     start=not acc, stop=True)
                ob = sbO.tile([128, C], F16, name="ob", tag="ob")
                if t == NT - 1:
                    # tail: split the drain across both engines to halve
                    # the serial drain->DMA chain of the last b-tiles
                    nc.scalar.activation(ob[:, :512], pl[:, :512],
                                         mybir.ActivationFunctionType.Copy,
                                         bias=0.0, scale=1.0)
                    nc.vector.tensor_copy(ob[:, 512:C], pl[:, 512:C])
                else:
                    nc.scalar.activation(ob[:], pl[:, :C],
                                         mybir.ActivationFunctionType.Copy,
                                         bias=0.0, scale=1.0)
                nc.sync.dma_start(out_d[t * 512 + bs_i * 128:
                                        t * 512 + (bs_i + 1) * 128, :], ob[:])
    nc.finalize()
    return nc


def kernel(features, group_probs, W, b, label_ids):
    global LAST_EXEC_NS
    features = np.asarray(features, dtype=np.float32)
    group_probs = np.asarray(group_probs, dtype=np.float32)
    prep = _host_prep(W, b, label_ids)
    nc = _build_program(prep["SSW"], prep["segs"])

    XT = np.ascontiguousarray(features.T.astype(np.float16))  # [F, B]
    PT = group_probs.T.astype(np.float16)                     # [G, B]
    gmap = prep["gmap"]
    in_maps = []
    for c in range(NCORE):
        ptc = PT[:, c * BC:(c + 1) * BC].reshape(G, NT, 512)  # [16, 8, 512]
        ptx = np.empty((NT, 128, NCH * 512), dtype=np.float16)
        for j in range(NCH):
            # [128, NT, 512] -> [NT, 128, 512]
            ptx[:, :, j * 512:(j + 1) * 512] = \
                ptc[gmap[j]].transpose(1, 0, 2)
        in_maps.append({
            "xt": np.ascontiguousarray(XT[:, c * BC:(c + 1) * BC]),
            "ptx": np.ascontiguousarray(ptx.reshape(NT * 128, NCH * 512)),
            "wt": prep["WT"],
            "bt": prep["biasT"],
            "s": prep["SS"],
        })

    trace = bool(os.environ.get("BASS_TRACE"))
    if trace:
        bass_utils.upload_artifacts = lambda d: "local://skipped"
    try:
        res = bass_utils.run_bass_kernel_spmd(nc, in_maps,
                                              core_ids=list(range(NCORE)))
    except Exception:
        # transient NRT device errors have been observed; one retry
        res = bass_utils.run_bass_kernel_spmd(nc, in_maps,
                                              core_ids=list(range(NCORE)))
    if trace:
        LAST_EXEC_NS = res.exec_time_ns
        if res.exec_time_ns is not None:
            print(f"HW exec time: {res.exec_time_ns} ns")

    out = np.concatenate([res.results[c]["logits"] for c in range(NCORE)],
                         axis=0)
    return np.ascontiguousarray(out.astype(np.float32))
